# revision 15
# baseline (speedup 1.0000x reference)
"""Multi-head attention (softmax over the HEADS axis) on 8 trn2 NeuronCores.

Reference math (B=2, S=2048, D=512, H=8, Dk=64):
    q = split_heads(Q @ w_q.T + b_q)          # [B,H,S,Dk]
    scores = q @ k.T / sqrt(Dk)               # [B,H,Sq,Sk]
    probs = softmax(scores, axis=1)           # softmax over H (source quirk!)
    attn = probs @ v                          # [B,H,Sq,Dk]
    out = concat_heads(attn) @ w_o.T + b_o    # [B,S,D]

Softmax over H is local to each (b, sq, sk) position: sharding over
(batch x query rows) needs no cross-core communication.  Core c handles
batch c//4, query rows (c%4)*512 .. +512, with all 8 heads resident.

Schedule: DMA is issued in dependency-priority order (wq,q / wk,k-chunks /
wv,v-chunks / wo) so the Q projection starts ~3us in and the first score
tile ~7us in.  K projection is emitted kc-major; kc1..3 and the V
projection pairs are interleaved into the attention loop so PE never has
a >1us bubble (HAM clock-gate stays at 8/8).  Score matmul pairs and attn
matmul pairs are emitted adjacently: they row/col-tile into disjoint PE
quadrant groups and overlap ~2x.

Steady state is elementwise-bound.  Measured: DVE tensor_tensor bf16
runs in 2x_1P mode (2 elem/cycle) ONLY when GpSimd is idle - any GpSimd
op degrades concurrent DVE ops 2-4x via the shared SBUF port.  So the
whole softmax chain lives on DVE (~5.3us/tile, all bf16 2x ops:
head-sum tree FD2048/FD1024, fp32 level-3 add, fast reciprocal, one
FD4096 broadcast normalization mul) and GpSimd is NOT used at all.
ACT does 4 exps [128,2,512] from PSUM (~4.0us/tile) plus all psum->sbuf
projection drains.  PSUM: 4 banks of score tiles (double-buffered,
shared with V/Q/K/O-proj pairs) + 4 banks of attn accumulators.
"""

import numpy as np

B, S, D, H, DK = 2, 2048, 512, 8, 64
NCORES = 8
CPB = NCORES // B          # cores per batch
QI = S // CPB              # query rows per core (512)
KJT = 128                  # kj tile (partition dim of scores)
NKJ = S // KJT             # 16 kj tiles
NC_, CH = 128, 4           # partitions, din chunks
NKC = 4                    # kj column chunks for K proj (512 each)
SCALE = 1.0 / np.sqrt(DK)  # folded into exp activation
LAG = 3                    # attn matmuls run LAG tiles behind softmax


def _chunk(x, dt):
    """[512, F] -> [128, 4, F] with row = chunk*128 + p."""
    f = x.shape[1]
    return np.ascontiguousarray(
        np.ascontiguousarray(x).reshape(CH, NC_, f).transpose(1, 0, 2)
    ).astype(dt)


def _build(with_bias):
    from contextlib import ExitStack

    import concourse.bass as bass
    import concourse.mybir as mybir
    import concourse.tile as tile
    from concourse import bacc
    from concourse.dve_ops import (
        RECIP_APPROX_FAST_CONSTS as _RC,
        RECIPROCAL_APPROX_FAST as _RF,
    )

    fp32 = mybir.dt.float32
    bf16 = mybir.dt.bfloat16
    EXP = mybir.ActivationFunctionType.Exp

    nc = bacc.Bacc(
        "TRN2",
        target_bir_lowering=False,
        debug=False,
        enable_asserts=False,
        num_devices=NCORES,
    )

    def din(name, shape):
        return nc.dram_tensor(name, shape, bf16, kind="ExternalInput").ap()

    qt_d = din("qt", [NC_, CH, QI])
    kt_d = din("kt", [NC_, NKC, CH, 512])    # kc-major so kc0 lands first
    vt_d = din("vt", [NC_, NKJ, CH, KJT])    # tile-major so early tiles land first
    id_d = din("ident", [NC_, NC_])          # identity for PE head-sum matmuls
    w_d = {n: din(n, [NC_, CH, D]) for n in ("wqt", "wkt", "wvt", "wot")}
    if with_bias:
        b_d = {n: din(n, [1, D]) for n in ("bq", "bk", "bv", "bo")}
    out_d = nc.dram_tensor("out", [QI, D], fp32, kind="ExternalOutput").ap()

    with tile.TileContext(nc) as tc, ExitStack() as ctx:
        wpool = ctx.enter_context(tc.tile_pool(name="wts", bufs=1))
        raw = ctx.enter_context(tc.tile_pool(name="raw", bufs=1))
        acts = ctx.enter_context(tc.tile_pool(name="acts", bufs=1))
        sm = ctx.enter_context(tc.tile_pool(name="sm", bufs=3))
        pp = ctx.enter_context(tc.tile_pool(name="pp", bufs=5))
        ps = ctx.enter_context(tc.tile_pool(name="ps", bufs=2, space="PSUM"))
        psa = ctx.enter_context(tc.tile_pool(name="psa", bufs=4, space="PSUM"))

        # ---------------- DMA in priority order ----------------
        wsb = {}
        for n in ("wqt", "wkt", "wvt", "wot"):
            wsb[n] = wpool.tile([NC_, CH, D], bf16, tag=n, name=n)
        qraw = raw.tile([NC_, CH, QI], bf16, tag="qraw")
        kraw = raw.tile([NC_, NKC, CH, 512], bf16, tag="kraw")
        vraw = raw.tile([NC_, NKJ, CH, KJT], bf16, tag="vraw")

        ident = acts.tile([NC_, NC_], bf16, tag="ident")
        nc.sync.dma_start(out=ident, in_=id_d)
        nc.sync.dma_start(out=wsb["wqt"], in_=w_d["wqt"])
        nc.sync.dma_start(out=qraw, in_=qt_d)
        nc.sync.dma_start(out=wsb["wkt"], in_=w_d["wkt"])
        nc.sync.dma_start(out=kraw[:, 0], in_=kt_d[:, 0])
        nc.sync.dma_start(out=kraw[:, 1], in_=kt_d[:, 1])
        nc.sync.dma_start(out=wsb["wvt"], in_=w_d["wvt"])
        nc.sync.dma_start(out=vraw[:, 0:8], in_=vt_d[:, 0:8])
        nc.sync.dma_start(out=kraw[:, 2], in_=kt_d[:, 2])
        nc.sync.dma_start(out=vraw[:, 8:16], in_=vt_d[:, 8:16])
        nc.sync.dma_start(out=kraw[:, 3], in_=kt_d[:, 3])
        nc.sync.dma_start(out=wsb["wot"], in_=w_d["wot"])

        if with_bias:
            ones = acts.tile([1, D], bf16, tag="ones")
            nc.vector.memset(ones, 1.0)
            brow = {}
            for n in ("bq", "bk", "bv", "bo"):
                brow[n] = acts.tile([1, D], bf16, tag=n, name=n)
                nc.sync.dma_start(out=brow[n], in_=b_d[n])

        def bias_mm(pt_ap, bname, col_slice):
            """rank-1 bias init: psum = bias-row (x) ones-row (or flipped)."""
            if col_slice is not None:  # bias along partitions
                lhsT = brow[bname][:, col_slice]
                rhs = ones[:, : pt_ap.shape[-1]]
            else:  # bias along free dim
                lhsT = ones[:, :128]
                rhs = brow[bname]
            nc.tensor.matmul(pt_ap, lhsT=lhsT, rhs=rhs, start=True, stop=False)

        qTs = acts.tile([NC_, CH, QI], bf16, tag="qTs")
        kTs = acts.tile([NC_, CH, S], bf16, tag="kTs")
        vs = acts.tile([NC_, NKJ, D], bf16, tag="vs")
        attnT = acts.tile([NC_, CH, QI], bf16, tag="attnT")
        outsb = acts.tile([NC_, CH, D], fp32, tag="outsb")

        # ---------------- Q projection ----------------
        for mh in range(2):  # two m per psum tile
            pt = ps.tile([NC_, 2, 512], fp32, tag="sc")
            for j in range(2):
                m = 2 * mh + j
                if with_bias:
                    bias_mm(pt[:, j, :QI], "bq", slice(m * 128, (m + 1) * 128))
                for c in range(CH):
                    nc.tensor.matmul(
                        pt[:, j, :QI],
                        lhsT=wsb["wqt"][:, c, m * 128 : (m + 1) * 128],
                        rhs=qraw[:, c, :],
                        start=(c == 0 and not with_bias),
                        stop=(c == CH - 1),
                    )
            nc.scalar.copy(qTs[:, 2 * mh : 2 * mh + 2, :], pt)

        def k_proj_mh(kc, mh):
            pt = ps.tile([NC_, 2, 512], fp32, tag="sc")
            for j in range(2):
                m = 2 * mh + j
                if with_bias:
                    bias_mm(pt[:, j, :], "bk", slice(m * 128, (m + 1) * 128))
                for c in range(CH):
                    nc.tensor.matmul(
                        pt[:, j, :],
                        lhsT=wsb["wkt"][:, c, m * 128 : (m + 1) * 128],
                        rhs=kraw[:, kc, c, :],
                        start=(c == 0 and not with_bias),
                        stop=(c == CH - 1),
                    )
            nc.scalar.copy(
                kTs[:, 2 * mh : 2 * mh + 2, kc * 512 : (kc + 1) * 512], pt
            )

        k_proj_mh(0, 0)
        k_proj_mh(0, 1)

        def v_proj_pair(i, on_dve):
            """project v tiles 2i, 2i+1 into vs (one psum tile)."""
            pt = ps.tile([NC_, 2, 512], fp32, tag="sc")
            for half in range(2):
                t = 2 * i + half
                if with_bias:
                    bias_mm(pt[:, half, :], "bv", None)
                for c in range(CH):
                    nc.tensor.matmul(
                        pt[:, half, :],
                        lhsT=vraw[:, t, c, :],
                        rhs=wsb["wvt"][:, c, :],
                        start=(c == 0 and not with_bias),
                        stop=(c == CH - 1),
                    )
            if on_dve:
                nc.vector.tensor_copy(vs[:, 2 * i : 2 * i + 2, :], pt[:, :, :])
            else:
                nc.scalar.copy(vs[:, 2 * i : 2 * i + 2, :], pt[:, :, :])

        # attn psum: tile dc holds heads 2dc (p 0..63), 2dc+1 (p 64..127)
        at = [psa.tile([NC_, 512], fp32, tag="attn", name=f"at{i}") for i in range(4)]

        def emit_attn(td, prs):
            for h in range(H):
                po = (h % 2) * 64
                nc.tensor.matmul(
                    at[h // 2][po : po + 64, :QI],
                    lhsT=vs[:, td, h * 64 : (h + 1) * 64],
                    rhs=prs[:, h, :],
                    start=(td == 0),
                    stop=(td == NKJ - 1),
                )

        # ---------------- attention loop ----------------
        pending = []

        def softmax_tail(tp, exp_p):
            """Head-sum on PE (identity-matmul accumulation of all 8 heads
            into one fp32 PSUM bank), then reciprocal (reads PSUM directly)
            + broadcast normalize on DVE."""
            sp = ps.tile([NC_, 2, 512], fp32, tag="sc")
            ssum = sp[:, 0, :]
            for h in range(H):
                nc.tensor.matmul(
                    ssum,
                    lhsT=ident,
                    rhs=exp_p[:, h, :],
                    start=(h == 0),
                    stop=(h == H - 1),
                )
            r = sm.tile([NC_, QI], bf16, tag="r")
            nc.vector._custom_dve(
                _RF, out=r, in0=ssum, s0=_RC["s0"], s1=_RC["s1"], imm2=_RC["imm2"]
            )
            rb = r.unsqueeze(1).broadcast_to([NC_, H, QI])
            pr = pp.tile([NC_, H, QI], bf16, tag="probs")
            nc.vector.tensor_mul(pr, exp_p, rb)
            pending.append((tp, pr))

        prev = None
        for t in range(NKJ):
            exp_t = sm.tile([NC_, H, QI], bf16, tag="exp")
            for m in range(4):
                spt = ps.tile([NC_, 2, 512], fp32, tag="sc")
                for j in range(2):
                    po = j * 64
                    nc.tensor.matmul(
                        spt[:, j, :QI],
                        lhsT=kTs[po : po + 64, m, t * 128 : (t + 1) * 128],
                        rhs=qTs[po : po + 64, m, :],
                        start=True,
                        stop=True,
                    )
                nc.scalar.activation(
                    exp_t[:, 2 * m : 2 * m + 2, :], spt, EXP, scale=SCALE
                )

            # softmax tail of the PREVIOUS tile: its exps are long done, so
            # the PE sum matmuls never head-of-line block the queue
            if prev is not None:
                softmax_tail(*prev)
            if len(pending) >= LAG:
                emit_attn(*pending.pop(0))
            # interleaved projection work, spread at half-kc / v-pair
            # granularity so no single tile gets a big PE or ACT lump;
            # V-proj drains go to DVE on tiles where ACT also has a K copy
            if 1 <= t <= 6:
                k_proj_mh(1 + (t - 1) // 2, (t - 1) % 2)
            if 2 <= t <= 9:
                v_proj_pair(t - 2, on_dve=(t <= 6))
            prev = (t, exp_t)

        softmax_tail(*prev)
        for td, prs in pending:
            emit_attn(td, prs)

        for dc in range(4):
            if dc % 2 == 0:
                nc.vector.tensor_copy(attnT[:, dc, :], at[dc][:, :QI])
            else:
                nc.scalar.copy(attnT[:, dc, :], at[dc][:, :QI])

        # ---------------- output projection ----------------
        for mh in range(2):
            ot = ps.tile([NC_, 2, 512], fp32, tag="sc")
            for j in range(2):
                m = 2 * mh + j
                if with_bias:
                    bias_mm(ot[:, j, :], "bo", None)
                for c in range(CH):
                    nc.tensor.matmul(
                        ot[:, j, :],
                        lhsT=attnT[:, c, m * 128 : (m + 1) * 128],
                        rhs=wsb["wot"][:, c, :],
                        start=(c == 0 and not with_bias),
                        stop=(c == CH - 1),
                    )
            if mh == 0:
                nc.scalar.copy(outsb[:, 0:2, :], ot)
            else:
                nc.vector.tensor_copy(outsb[:, 2:4, :], ot)
            nc.sync.dma_start(
                out=out_d.rearrange("(m p) o -> p m o", p=NC_)[:, 2 * mh : 2 * mh + 2, :],
                in_=outsb[:, 2 * mh : 2 * mh + 2, :],
            )

    nc.compile()
    return nc


_CACHE = {}


def kernel(Q, K, V, w_q, b_q, w_k, b_k, w_v, b_v, w_o, b_o, _trace=False):
    import ml_dtypes
    from concourse import bass_utils

    bf = ml_dtypes.bfloat16
    Q = np.asarray(Q, np.float32)
    K = np.asarray(K, np.float32)
    V = np.asarray(V, np.float32)
    with_bias = any(
        np.any(np.asarray(b) != 0) for b in (b_q, b_k, b_v, b_o)
    )

    if ("nc", with_bias) not in _CACHE:
        _CACHE[("nc", with_bias)] = _build(with_bias)
    nc = _CACHE[("nc", with_bias)]

    wmaps = {
        "wqt": _chunk(np.asarray(w_q, np.float32).T, bf),
        "wkt": _chunk(np.asarray(w_k, np.float32).T, bf),
        "wvt": _chunk(np.asarray(w_v, np.float32).T, bf),
        "wot": _chunk(np.asarray(w_o, np.float32).T, bf),
        "ident": np.eye(NC_, dtype=bf),
    }
    if with_bias:
        for n, b in (("bq", b_q), ("bk", b_k), ("bv", b_v), ("bo", b_o)):
            wmaps[n] = np.ascontiguousarray(
                np.asarray(b, np.float32).reshape(1, D)
            ).astype(bf)

    in_maps = []
    for c in range(NCORES):
        b = c // CPB
        s0 = (c % CPB) * QI
        kt = _chunk(K[b].T, bf)                   # [128, 4c, 2048]
        vt = _chunk(V[b].T, bf)
        in_maps.append(
            dict(
                wmaps,
                qt=_chunk(Q[b, s0 : s0 + QI, :].T, bf),
                # [128, c, kc*512] -> [128, kc, c, 512]
                kt=np.ascontiguousarray(
                    kt.reshape(NC_, CH, NKC, 512).transpose(0, 2, 1, 3)
                ),
                # [128, c, t*128] -> [128, t, c, 128]
                vt=np.ascontiguousarray(
                    vt.reshape(NC_, CH, NKJ, KJT).transpose(0, 2, 1, 3)
                ),
            )
        )

    res = bass_utils.run_bass_kernel_spmd(
        nc, in_maps, core_ids=list(range(NCORES)), trace=_trace
    )

    out = np.empty((B, S, D), np.float32)
    for c in range(NCORES):
        b = c // CPB
        s0 = (c % CPB) * QI
        out[b, s0 : s0 + QI, :] = res.results[c]["out"]
    if _trace:
        kernel._last_results = res
    return out


# revision 20
# speedup vs baseline: 1.0151x; 1.0151x over previous
"""Multi-head attention (softmax over the HEADS axis) on 8 trn2 NeuronCores.

Reference math (B=2, S=2048, D=512, H=8, Dk=64):
    q = split_heads(Q @ w_q.T + b_q)          # [B,H,S,Dk]
    scores = q @ k.T / sqrt(Dk)               # [B,H,Sq,Sk]
    probs = softmax(scores, axis=1)           # softmax over H (source quirk!)
    attn = probs @ v                          # [B,H,Sq,Dk]
    out = concat_heads(attn) @ w_o.T + b_o    # [B,S,D]

Softmax over H is local to each (b, sq, sk) position: sharding over
(batch x query rows) needs no cross-core communication.  Core c handles
batch c//4, query rows (c%4)*512 .. +512, with all 8 heads resident.

Schedule: DMA is issued in dependency-priority order (wq,q / wk,k-chunks /
wv,v-chunks / wo) so the Q projection starts ~3us in and the first score
tile ~7us in.  K projection is emitted kc-major; kc1..3 and the V
projection pairs are interleaved into the attention loop so PE never has
a >1us bubble (HAM clock-gate stays at 8/8).  Score matmul pairs and attn
matmul pairs are emitted adjacently: they row/col-tile into disjoint PE
quadrant groups and overlap ~2x.

Steady state is elementwise-bound.  Measured: DVE tensor_tensor bf16
runs in 2x_1P mode (2 elem/cycle) ONLY when GpSimd is idle - any GpSimd
op degrades concurrent DVE ops 2-4x via the shared SBUF port.  So the
whole softmax chain lives on DVE (~5.3us/tile, all bf16 2x ops:
head-sum tree FD2048/FD1024, fp32 level-3 add, fast reciprocal, one
FD4096 broadcast normalization mul) and GpSimd is NOT used at all.
ACT does 4 exps [128,2,512] from PSUM (~4.0us/tile) plus all psum->sbuf
projection drains.  PSUM: 4 banks of score tiles (double-buffered,
shared with V/Q/K/O-proj pairs) + 4 banks of attn accumulators.
"""

import numpy as np

B, S, D, H, DK = 2, 2048, 512, 8, 64
NCORES = 8
CPB = NCORES // B          # cores per batch
QI = S // CPB              # query rows per core (512)
KJT = 128                  # kj tile (partition dim of scores)
NKJ = S // KJT             # 16 kj tiles
NC_, CH = 128, 4           # partitions, din chunks
NKC = 4                    # kj column chunks for K proj (512 each)
SCALE = 1.0 / np.sqrt(DK)  # folded into exp activation
LAG = 3                    # attn matmuls run LAG tiles behind softmax


def _chunk(x, dt):
    """[512, F] -> [128, 4, F] with row = chunk*128 + p."""
    f = x.shape[1]
    return np.ascontiguousarray(
        np.ascontiguousarray(x).reshape(CH, NC_, f).transpose(1, 0, 2)
    ).astype(dt)


def _build(with_bias):
    from contextlib import ExitStack

    import concourse.bass as bass
    import concourse.mybir as mybir
    import concourse.tile as tile
    from concourse import bacc
    from concourse.dve_ops import (
        RECIP_APPROX_FAST_CONSTS as _RC,
        RECIPROCAL_APPROX_FAST as _RF,
    )

    fp32 = mybir.dt.float32
    bf16 = mybir.dt.bfloat16
    EXP = mybir.ActivationFunctionType.Exp

    nc = bacc.Bacc(
        "TRN2",
        target_bir_lowering=False,
        debug=False,
        enable_asserts=False,
        num_devices=NCORES,
    )

    def din(name, shape):
        return nc.dram_tensor(name, shape, bf16, kind="ExternalInput").ap()

    qt_d = din("qt", [NC_, CH, QI])
    kt_d = din("kt", [NC_, NKC, CH, 512])    # kc-major so kc0 lands first
    vt_d = din("vt", [NC_, NKJ, CH, KJT])    # tile-major so early tiles land first
    id_d = din("ident", [NC_, NC_])          # identity for PE head-sum matmuls
    w_d = {n: din(n, [NC_, CH, D]) for n in ("wqt", "wkt", "wvt", "wot")}
    if with_bias:
        b_d = {n: din(n, [1, D]) for n in ("bq", "bk", "bv", "bo")}
    out_d = nc.dram_tensor("out", [QI, D], fp32, kind="ExternalOutput").ap()

    with tile.TileContext(nc) as tc, ExitStack() as ctx:
        wpool = ctx.enter_context(tc.tile_pool(name="wts", bufs=1))
        raw = ctx.enter_context(tc.tile_pool(name="raw", bufs=1))
        acts = ctx.enter_context(tc.tile_pool(name="acts", bufs=1))
        sm = ctx.enter_context(tc.tile_pool(name="sm", bufs=3))
        pp = ctx.enter_context(tc.tile_pool(name="pp", bufs=5))
        ps = ctx.enter_context(tc.tile_pool(name="ps", bufs=2, space="PSUM"))
        psa = ctx.enter_context(tc.tile_pool(name="psa", bufs=4, space="PSUM"))

        # ---------------- DMA in priority order ----------------
        wsb = {}
        for n in ("wqt", "wkt", "wvt", "wot"):
            wsb[n] = wpool.tile([NC_, CH, D], bf16, tag=n, name=n)
        qraw = raw.tile([NC_, CH, QI], bf16, tag="qraw")
        kraw = raw.tile([NC_, NKC, CH, 512], bf16, tag="kraw")
        vraw = raw.tile([NC_, NKJ, CH, KJT], bf16, tag="vraw")

        ident = acts.tile([NC_, NC_], bf16, tag="ident")
        nc.sync.dma_start(out=ident, in_=id_d)
        nc.sync.dma_start(out=wsb["wqt"], in_=w_d["wqt"])
        nc.sync.dma_start(out=qraw, in_=qt_d)
        nc.sync.dma_start(out=wsb["wkt"], in_=w_d["wkt"])
        nc.sync.dma_start(out=kraw[:, 0], in_=kt_d[:, 0])
        nc.sync.dma_start(out=kraw[:, 1], in_=kt_d[:, 1])
        nc.sync.dma_start(out=wsb["wvt"], in_=w_d["wvt"])
        nc.sync.dma_start(out=vraw[:, 0:8], in_=vt_d[:, 0:8])
        nc.sync.dma_start(out=kraw[:, 2], in_=kt_d[:, 2])
        nc.sync.dma_start(out=vraw[:, 8:16], in_=vt_d[:, 8:16])
        nc.sync.dma_start(out=kraw[:, 3], in_=kt_d[:, 3])
        nc.sync.dma_start(out=wsb["wot"], in_=w_d["wot"])

        if with_bias:
            ones = acts.tile([1, D], bf16, tag="ones")
            nc.vector.memset(ones, 1.0)
            brow = {}
            for n in ("bq", "bk", "bv", "bo"):
                brow[n] = acts.tile([1, D], bf16, tag=n, name=n)
                nc.sync.dma_start(out=brow[n], in_=b_d[n])

        def bias_mm(pt_ap, bname, col_slice):
            """rank-1 bias init: psum = bias-row (x) ones-row (or flipped)."""
            if col_slice is not None:  # bias along partitions
                lhsT = brow[bname][:, col_slice]
                rhs = ones[:, : pt_ap.shape[-1]]
            else:  # bias along free dim
                lhsT = ones[:, :128]
                rhs = brow[bname]
            nc.tensor.matmul(pt_ap, lhsT=lhsT, rhs=rhs, start=True, stop=False)

        qTs = acts.tile([NC_, CH, QI], bf16, tag="qTs")
        kTs = acts.tile([NC_, CH, S], bf16, tag="kTs")
        vs = acts.tile([NC_, NKJ, D], bf16, tag="vs")
        attnT = acts.tile([NC_, CH, QI], bf16, tag="attnT")
        outsb = acts.tile([NC_, CH, D], fp32, tag="outsb")

        # attn psum accumulators are declared early so the PE warm-up can
        # scribble into at[0] (attn td=0 uses start=True, overwriting it)
        at = [psa.tile([NC_, 512], fp32, tag="attn", name=f"at{i}") for i in range(4)]

        # PE warm-up: ~48 tiny matmuls as soon as `ident` lands, while the
        # big input DMAs stream.  Sustained PE activity releases the HAM
        # clock gate (4/8 -> 8/8) so the projections run at 2.4 GHz instead
        # of 1.2 GHz.  Results are garbage; attn td=0 overwrites at[0].
        for i in range(48):
            nc.tensor.matmul(
                at[0][:, 0:128], lhsT=ident, rhs=ident,
                start=True, stop=True, skip_group_check=True,
            )

        # ---------------- Q projection ----------------
        for mh in range(2):  # two m per psum tile
            pt = ps.tile([NC_, 2, 512], fp32, tag="sc")
            for j in range(2):
                m = 2 * mh + j
                if with_bias:
                    bias_mm(pt[:, j, :QI], "bq", slice(m * 128, (m + 1) * 128))
                for c in range(CH):
                    nc.tensor.matmul(
                        pt[:, j, :QI],
                        lhsT=wsb["wqt"][:, c, m * 128 : (m + 1) * 128],
                        rhs=qraw[:, c, :],
                        start=(c == 0 and not with_bias),
                        stop=(c == CH - 1),
                    )
            nc.scalar.copy(qTs[:, 2 * mh : 2 * mh + 2, :], pt)

        def k_proj_mh(kc, mh):
            pt = ps.tile([NC_, 2, 512], fp32, tag="sc")
            for j in range(2):
                m = 2 * mh + j
                if with_bias:
                    bias_mm(pt[:, j, :], "bk", slice(m * 128, (m + 1) * 128))
                for c in range(CH):
                    nc.tensor.matmul(
                        pt[:, j, :],
                        lhsT=wsb["wkt"][:, c, m * 128 : (m + 1) * 128],
                        rhs=kraw[:, kc, c, :],
                        start=(c == 0 and not with_bias),
                        stop=(c == CH - 1),
                    )
            nc.scalar.copy(
                kTs[:, 2 * mh : 2 * mh + 2, kc * 512 : (kc + 1) * 512], pt
            )

        k_proj_mh(0, 0)
        k_proj_mh(0, 1)

        def v_proj_pair(i, on_dve):
            """project v tiles 2i, 2i+1 into vs (one psum tile)."""
            pt = ps.tile([NC_, 2, 512], fp32, tag="sc")
            for half in range(2):
                t = 2 * i + half
                if with_bias:
                    bias_mm(pt[:, half, :], "bv", None)
                for c in range(CH):
                    nc.tensor.matmul(
                        pt[:, half, :],
                        lhsT=vraw[:, t, c, :],
                        rhs=wsb["wvt"][:, c, :],
                        start=(c == 0 and not with_bias),
                        stop=(c == CH - 1),
                    )
            if on_dve:
                nc.vector.tensor_copy(vs[:, 2 * i : 2 * i + 2, :], pt[:, :, :])
            else:
                nc.scalar.copy(vs[:, 2 * i : 2 * i + 2, :], pt[:, :, :])

        # attn psum: tile dc holds heads 2dc (p 0..63), 2dc+1 (p 64..127)
        def emit_attn(td, prs):
            for h in range(H):
                po = (h % 2) * 64
                nc.tensor.matmul(
                    at[h // 2][po : po + 64, :QI],
                    lhsT=vs[:, td, h * 64 : (h + 1) * 64],
                    rhs=prs[:, h, :],
                    start=(td == 0),
                    stop=(td == NKJ - 1),
                )

        # ---------------- attention loop ----------------
        pending = []

        def softmax_tail(tp, exp_p):
            """Head-sum: levels 1-2 on DVE (bf16 2x_1P adds), final pair
            summed on PE by two identity matmuls into a PSUM bank; the fast
            reciprocal reads that PSUM directly (one PSUM input is legal),
            then one FD4096 broadcast normalize mul on DVE."""
            a1 = sm.tile([NC_, 4, QI], bf16, tag="a1")
            nc.vector.tensor_add(a1[:, 0:2, :], exp_p[:, 0:2, :], exp_p[:, 2:4, :])
            nc.vector.tensor_add(a1[:, 2:4, :], exp_p[:, 4:6, :], exp_p[:, 6:8, :])
            a2 = sm.tile([NC_, 2, QI], bf16, tag="a2")
            nc.vector.tensor_add(a2, a1[:, 0:2, :], a1[:, 2:4, :])
            sp = ps.tile([NC_, 2, 512], fp32, tag="sc")
            ssum = sp[:, 0, :]
            for j in range(2):
                nc.tensor.matmul(
                    ssum,
                    lhsT=ident,
                    rhs=a2[:, j, :],
                    start=(j == 0),
                    stop=(j == 1),
                )
            r = sm.tile([NC_, QI], bf16, tag="r")
            nc.vector._custom_dve(
                _RF, out=r, in0=ssum, s0=_RC["s0"], s1=_RC["s1"], imm2=_RC["imm2"]
            )
            rb = r.unsqueeze(1).broadcast_to([NC_, H, QI])
            pr = pp.tile([NC_, H, QI], bf16, tag="probs")
            nc.vector.tensor_mul(pr, exp_p, rb)
            pending.append((tp, pr))

        prev = None
        for t in range(NKJ):
            exp_t = sm.tile([NC_, H, QI], bf16, tag="exp")
            for m in range(4):
                spt = ps.tile([NC_, 2, 512], fp32, tag="sc")
                for j in range(2):
                    po = j * 64
                    nc.tensor.matmul(
                        spt[:, j, :QI],
                        lhsT=kTs[po : po + 64, m, t * 128 : (t + 1) * 128],
                        rhs=qTs[po : po + 64, m, :],
                        start=True,
                        stop=True,
                    )
                nc.scalar.activation(
                    exp_t[:, 2 * m : 2 * m + 2, :], spt, EXP, scale=SCALE
                )

            # softmax tail of the PREVIOUS tile: its exps are long done, so
            # the PE sum matmuls never head-of-line block the queue
            if prev is not None:
                softmax_tail(*prev)
            if len(pending) >= LAG:
                emit_attn(*pending.pop(0))
            # interleaved projection work: exactly ONE half-kc or v-pair
            # unit per tile (~1.7us PE + ~1us ACT copy) so no tile gets a
            # lump; deadlines: kc_n before tile 4n, v-pair i before
            # attn(2i) at loop 2i+4
            KP = {2: (1, 0), 3: (1, 1), 6: (2, 0), 7: (2, 1), 10: (3, 0), 11: (3, 1)}
            VP = {1: 0, 4: 1, 5: 2, 8: 3, 9: 4, 12: 5, 13: 6, 14: 7}
            if t in KP:
                k_proj_mh(*KP[t])
            if t in VP:
                v_proj_pair(VP[t], on_dve=False)
            prev = (t, exp_t)

        # drain attn whose probs already exist BEFORE the last softmax tail,
        # so PE chews the backlog while DVE finishes tile 15's chain
        while pending:
            emit_attn(*pending.pop(0))
        softmax_tail(*prev)
        while pending:
            emit_attn(*pending.pop(0))

        for dc in range(4):
            if dc % 2 == 0:
                nc.vector.tensor_copy(attnT[:, dc, :], at[dc][:, :QI])
            else:
                nc.scalar.copy(attnT[:, dc, :], at[dc][:, :QI])

        # ---------------- output projection ----------------
        for mh in range(2):
            ot = ps.tile([NC_, 2, 512], fp32, tag="sc")
            for j in range(2):
                m = 2 * mh + j
                if with_bias:
                    bias_mm(ot[:, j, :], "bo", None)
                for c in range(CH):
                    nc.tensor.matmul(
                        ot[:, j, :],
                        lhsT=attnT[:, c, m * 128 : (m + 1) * 128],
                        rhs=wsb["wot"][:, c, :],
                        start=(c == 0 and not with_bias),
                        stop=(c == CH - 1),
                    )
            if mh == 0:
                nc.scalar.copy(outsb[:, 0:2, :], ot)
            else:
                nc.vector.tensor_copy(outsb[:, 2:4, :], ot)
            nc.sync.dma_start(
                out=out_d.rearrange("(m p) o -> p m o", p=NC_)[:, 2 * mh : 2 * mh + 2, :],
                in_=outsb[:, 2 * mh : 2 * mh + 2, :],
            )

    nc.compile()
    return nc


_CACHE = {}


def kernel(Q, K, V, w_q, b_q, w_k, b_k, w_v, b_v, w_o, b_o, _trace=False):
    import ml_dtypes
    from concourse import bass_utils

    bf = ml_dtypes.bfloat16
    Q = np.asarray(Q, np.float32)
    K = np.asarray(K, np.float32)
    V = np.asarray(V, np.float32)
    with_bias = any(
        np.any(np.asarray(b) != 0) for b in (b_q, b_k, b_v, b_o)
    )

    if ("nc", with_bias) not in _CACHE:
        _CACHE[("nc", with_bias)] = _build(with_bias)
    nc = _CACHE[("nc", with_bias)]

    wmaps = {
        "wqt": _chunk(np.asarray(w_q, np.float32).T, bf),
        "wkt": _chunk(np.asarray(w_k, np.float32).T, bf),
        "wvt": _chunk(np.asarray(w_v, np.float32).T, bf),
        "wot": _chunk(np.asarray(w_o, np.float32).T, bf),
        "ident": np.eye(NC_, dtype=bf),
    }
    if with_bias:
        for n, b in (("bq", b_q), ("bk", b_k), ("bv", b_v), ("bo", b_o)):
            wmaps[n] = np.ascontiguousarray(
                np.asarray(b, np.float32).reshape(1, D)
            ).astype(bf)

    in_maps = []
    for c in range(NCORES):
        b = c // CPB
        s0 = (c % CPB) * QI
        kt = _chunk(K[b].T, bf)                   # [128, 4c, 2048]
        vt = _chunk(V[b].T, bf)
        in_maps.append(
            dict(
                wmaps,
                qt=_chunk(Q[b, s0 : s0 + QI, :].T, bf),
                # [128, c, kc*512] -> [128, kc, c, 512]
                kt=np.ascontiguousarray(
                    kt.reshape(NC_, CH, NKC, 512).transpose(0, 2, 1, 3)
                ),
                # [128, c, t*128] -> [128, t, c, 128]
                vt=np.ascontiguousarray(
                    vt.reshape(NC_, CH, NKJ, KJT).transpose(0, 2, 1, 3)
                ),
            )
        )

    res = bass_utils.run_bass_kernel_spmd(
        nc, in_maps, core_ids=list(range(NCORES)), trace=_trace
    )

    out = np.empty((B, S, D), np.float32)
    for c in range(NCORES):
        b = c // CPB
        s0 = (c % CPB) * QI
        out[b, s0 : s0 + QI, :] = res.results[c]["out"]
    if _trace:
        kernel._last_results = res
    return out


# revision 28
# speedup vs baseline: 1.1507x; 1.1336x over previous
"""Multi-head attention (softmax over the HEADS axis) on 8 trn2 NeuronCores.

Reference math (B=2, S=2048, D=512, H=8, Dk=64):
    q = split_heads(Q @ w_q.T + b_q)          # [B,H,S,Dk]
    scores = q @ k.T / sqrt(Dk)               # [B,H,Sq,Sk]
    probs = softmax(scores, axis=1)           # softmax over H (source quirk!)
    attn = probs @ v                          # [B,H,Sq,Dk]
    out = concat_heads(attn) @ w_o.T + b_o    # [B,S,D]

Softmax over H is local to each (b, sq, sk) position: sharding over
(batch x query rows) needs no cross-core communication.  Core c handles
batch c//4, query rows (c%4)*512 .. +512, with all 8 heads resident.

Schedule: DMA is issued in dependency-priority order (wq,q / wk,k-chunks /
wv,v-chunks / wo) so the Q projection starts ~3us in and the first score
tile ~7us in.  K projection is emitted kc-major; kc1..3 and the V
projection pairs are interleaved into the attention loop so PE never has
a >1us bubble (HAM clock-gate stays at 8/8).  Score matmul pairs and attn
matmul pairs are emitted adjacently: they row/col-tile into disjoint PE
quadrant groups and overlap ~2x.

Steady state is elementwise-bound.  Measured: DVE tensor_tensor bf16
runs in 2x_1P mode (2 elem/cycle) ONLY when GpSimd is idle - any GpSimd
op degrades concurrent DVE ops 2-4x via the shared SBUF port.  So the
whole softmax chain lives on DVE (~5.3us/tile, all bf16 2x ops:
head-sum tree FD2048/FD1024, fp32 level-3 add, fast reciprocal, one
FD4096 broadcast normalization mul) and GpSimd is NOT used at all.
ACT does 4 exps [128,2,512] from PSUM (~4.0us/tile) plus all psum->sbuf
projection drains.  PSUM: 4 banks of score tiles (double-buffered,
shared with V/Q/K/O-proj pairs) + 4 banks of attn accumulators.
"""

import numpy as np

B, S, D, H, DK = 2, 2048, 512, 8, 64
NCORES = 8
CPB = NCORES // B          # cores per batch
QI = S // CPB              # query rows per core (512)
KJT = 128                  # kj tile (partition dim of scores)
NKJ = S // KJT             # 16 kj tiles
NC_, CH = 128, 4           # partitions, din chunks
NKC = 4                    # kj column chunks for K proj (512 each)
SCALE = 1.0 / np.sqrt(DK)  # folded into exp activation
LAG = 3                    # attn matmuls run LAG tiles behind softmax


def _chunk(x, dt):
    """[512, F] -> [128, 4, F] with row = chunk*128 + p."""
    f = x.shape[1]
    return np.ascontiguousarray(
        np.ascontiguousarray(x).reshape(CH, NC_, f).transpose(1, 0, 2)
    ).astype(dt)


def _build(with_bias):
    from contextlib import ExitStack

    import concourse.bass as bass
    import concourse.mybir as mybir
    import concourse.tile as tile
    from concourse import bacc
    from concourse.dve_ops import (
        RECIP_APPROX_FAST_CONSTS as _RC,
        RECIPROCAL_APPROX_FAST as _RF,
    )

    fp32 = mybir.dt.float32
    bf16 = mybir.dt.bfloat16
    EXP = mybir.ActivationFunctionType.Exp

    nc = bacc.Bacc(
        "TRN2",
        target_bir_lowering=False,
        debug=False,
        enable_asserts=False,
        num_devices=NCORES,
    )

    def din(name, shape):
        return nc.dram_tensor(name, shape, bf16, kind="ExternalInput").ap()

    qt_d = din("qt", [NC_, CH, QI])
    kt_d = din("kt", [NC_, NKC, CH, 512])    # kc-major so kc0 lands first
    vt_d = din("vt", [NC_, NKJ, CH, KJT])    # tile-major so early tiles land first
    id_d = din("ident", [NC_, NC_])          # identity for PE head-sum matmuls
    w_d = {n: din(n, [NC_, CH, D]) for n in ("wqt", "wkt", "wvt", "wot")}
    if with_bias:
        b_d = {n: din(n, [1, D]) for n in ("bq", "bk", "bv", "bo")}
    out_d = nc.dram_tensor("out", [QI, D], fp32, kind="ExternalOutput").ap()

    with tile.TileContext(nc) as tc, ExitStack() as ctx:
        wpool = ctx.enter_context(tc.tile_pool(name="wts", bufs=1))
        raw = ctx.enter_context(tc.tile_pool(name="raw", bufs=1))
        acts = ctx.enter_context(tc.tile_pool(name="acts", bufs=1))
        sm = ctx.enter_context(tc.tile_pool(name="sm", bufs=3))
        pp = ctx.enter_context(tc.tile_pool(name="pp", bufs=5))
        ps = ctx.enter_context(tc.tile_pool(name="ps", bufs=2, space="PSUM"))
        psa = ctx.enter_context(tc.tile_pool(name="psa", bufs=4, space="PSUM"))

        # ---------------- DMA in priority order ----------------
        wsb = {}
        for n in ("wqt", "wkt", "wvt", "wot"):
            wsb[n] = wpool.tile([NC_, CH, D], bf16, tag=n, name=n)
        qraw = raw.tile([NC_, CH, QI], bf16, tag="qraw")
        kraw = raw.tile([NC_, NKC, CH, 512], bf16, tag="kraw")
        vraw = raw.tile([NC_, NKJ, CH, KJT], bf16, tag="vraw")

        ident = acts.tile([NC_, NC_], bf16, tag="ident")
        nc.sync.dma_start(out=ident, in_=id_d)
        nc.sync.dma_start(out=wsb["wqt"], in_=w_d["wqt"])
        nc.sync.dma_start(out=qraw, in_=qt_d)
        nc.sync.dma_start(out=wsb["wkt"], in_=w_d["wkt"])
        nc.sync.dma_start(out=kraw[:, 0], in_=kt_d[:, 0])
        nc.sync.dma_start(out=kraw[:, 1], in_=kt_d[:, 1])
        nc.sync.dma_start(out=wsb["wvt"], in_=w_d["wvt"])
        nc.sync.dma_start(out=vraw[:, 0:8], in_=vt_d[:, 0:8])
        nc.sync.dma_start(out=kraw[:, 2], in_=kt_d[:, 2])
        nc.sync.dma_start(out=vraw[:, 8:16], in_=vt_d[:, 8:16])
        nc.sync.dma_start(out=kraw[:, 3], in_=kt_d[:, 3])
        nc.sync.dma_start(out=wsb["wot"], in_=w_d["wot"])

        if with_bias:
            ones = acts.tile([1, D], bf16, tag="ones")
            nc.vector.memset(ones, 1.0)
            brow = {}
            for n in ("bq", "bk", "bv", "bo"):
                brow[n] = acts.tile([1, D], bf16, tag=n, name=n)
                nc.sync.dma_start(out=brow[n], in_=b_d[n])

        def bias_mm(pt_ap, bname, col_slice):
            """rank-1 bias init: psum = bias-row (x) ones-row (or flipped)."""
            if col_slice is not None:  # bias along partitions
                lhsT = brow[bname][:, col_slice]
                rhs = ones[:, : pt_ap.shape[-1]]
            else:  # bias along free dim
                lhsT = ones[:, :128]
                rhs = brow[bname]
            nc.tensor.matmul(pt_ap, lhsT=lhsT, rhs=rhs, start=True, stop=False)

        qTs = acts.tile([NC_, CH, QI], bf16, tag="qTs")
        kTs = acts.tile([NC_, CH, S], bf16, tag="kTs")
        vs = acts.tile([NC_, NKJ, D], bf16, tag="vs")
        attnT = acts.tile([NC_, CH, QI], bf16, tag="attnT")
        outsb = acts.tile([NC_, CH, D], fp32, tag="outsb")

        at = [psa.tile([NC_, 512], fp32, tag="attn", name=f"at{i}") for i in range(4)]

        # PE warm-up: ~48 tiny matmuls as soon as `ident` lands, while the
        # big input DMAs stream.  Sustained PE activity releases the HAM
        # clock gate (4/8 -> 8/8) so the projections run at 2.4 GHz instead
        # of 1.2 GHz.  The throwaway psum tile has no readers and rotates
        # back into the scores pool, whose users all write with start=True.
        warm = ps.tile([NC_, 2, 512], fp32, tag="sc")
        for i in range(48):
            nc.tensor.matmul(
                warm[:, 0, 0:128], lhsT=ident, rhs=ident,
                start=True, stop=True, skip_group_check=True,
            )

        # ---------------- Q projection ----------------
        for mh in range(2):  # two m per psum tile
            pt = ps.tile([NC_, 2, 512], fp32, tag="sc")
            for j in range(2):
                m = 2 * mh + j
                if with_bias:
                    bias_mm(pt[:, j, :QI], "bq", slice(m * 128, (m + 1) * 128))
                for c in range(CH):
                    nc.tensor.matmul(
                        pt[:, j, :QI],
                        lhsT=wsb["wqt"][:, c, m * 128 : (m + 1) * 128],
                        rhs=qraw[:, c, :],
                        start=(c == 0 and not with_bias),
                        stop=(c == CH - 1),
                    )
            nc.scalar.copy(qTs[:, 2 * mh : 2 * mh + 2, :], pt)

        def k_proj_mh(kc, mh):
            pt = ps.tile([NC_, 2, 512], fp32, tag="sc")
            for j in range(2):
                m = 2 * mh + j
                if with_bias:
                    bias_mm(pt[:, j, :], "bk", slice(m * 128, (m + 1) * 128))
                for c in range(CH):
                    nc.tensor.matmul(
                        pt[:, j, :],
                        lhsT=wsb["wkt"][:, c, m * 128 : (m + 1) * 128],
                        rhs=kraw[:, kc, c, :],
                        start=(c == 0 and not with_bias),
                        stop=(c == CH - 1),
                    )
            nc.scalar.copy(
                kTs[:, 2 * mh : 2 * mh + 2, kc * 512 : (kc + 1) * 512], pt
            )

        k_proj_mh(0, 0)
        k_proj_mh(0, 1)

        def v_proj_pair(i, on_dve):
            """project v tiles 2i, 2i+1 into vs (one psum tile)."""
            pt = ps.tile([NC_, 2, 512], fp32, tag="sc")
            for half in range(2):
                t = 2 * i + half
                if with_bias:
                    bias_mm(pt[:, half, :], "bv", None)
                for c in range(CH):
                    nc.tensor.matmul(
                        pt[:, half, :],
                        lhsT=vraw[:, t, c, :],
                        rhs=wsb["wvt"][:, c, :],
                        start=(c == 0 and not with_bias),
                        stop=(c == CH - 1),
                    )
            if on_dve:
                nc.vector.tensor_copy(vs[:, 2 * i : 2 * i + 2, :], pt[:, :, :])
            else:
                nc.scalar.copy(vs[:, 2 * i : 2 * i + 2, :], pt[:, :, :])

        # attn psum: tile dc holds heads 2dc (p 0..63), 2dc+1 (p 64..127)
        def emit_attn(td, prs):
            for h in range(H):
                po = (h % 2) * 64
                nc.tensor.matmul(
                    at[h // 2][po : po + 64, :QI],
                    lhsT=vs[:, td, h * 64 : (h + 1) * 64],
                    rhs=prs[:, h, :],
                    start=(td == 0),
                    stop=(td == NKJ - 1),
                )

        # ---------------- attention loop ----------------
        pending = []

        def softmax_tail(tp, exp_p):
            """Head-sum tree + recip + normalize, all on DVE: bf16
            tensor_tensor hits 2x_1P mode (GpSimd stays idle to preserve
            it - shared SBUF port).  Level 1 is split so its first half
            only needs exps m0/m1 and runs in the shadow of exps m2/m3."""
            a1 = sm.tile([NC_, 4, QI], bf16, tag="a1")
            nc.vector.tensor_add(a1[:, 0:2, :], exp_p[:, 0:2, :], exp_p[:, 2:4, :])
            nc.vector.tensor_add(a1[:, 2:4, :], exp_p[:, 4:6, :], exp_p[:, 6:8, :])
            a2 = sm.tile([NC_, 2, QI], bf16, tag="a2")
            nc.vector.tensor_add(a2, a1[:, 0:2, :], a1[:, 2:4, :])
            ssum = sm.tile([NC_, QI], fp32, tag="ssum")
            nc.vector.tensor_add(ssum, a2[:, 0, :], a2[:, 1, :])
            r = sm.tile([NC_, QI], bf16, tag="r")
            nc.vector._custom_dve(
                _RF, out=r, in0=ssum, s0=_RC["s0"], s1=_RC["s1"], imm2=_RC["imm2"]
            )
            rb = r.unsqueeze(1).broadcast_to([NC_, H, QI])
            pr = pp.tile([NC_, H, QI], bf16, tag="probs")
            nc.vector.tensor_mul(pr, exp_p, rb)
            pending.append((tp, pr))

        for t in range(NKJ):
            exp_t = sm.tile([NC_, H, QI], bf16, tag="exp")
            for m in range(4):
                spt = ps.tile([NC_, 2, 512], fp32, tag="sc")
                for j in range(2):
                    po = j * 64
                    nc.tensor.matmul(
                        spt[:, j, :QI],
                        lhsT=kTs[po : po + 64, m, t * 128 : (t + 1) * 128],
                        rhs=qTs[po : po + 64, m, :],
                        start=True,
                        stop=True,
                    )
                nc.scalar.activation(
                    exp_t[:, 2 * m : 2 * m + 2, :], spt, EXP, scale=SCALE
                )

            # pop BEFORE this tile's append: attn(td) emits at iter td+3,
            # strictly after the v-pair writing vs[td] (VP deadlines below)
            if len(pending) >= LAG:
                emit_attn(*pending.pop(0))
            softmax_tail(t, exp_t)
            # interleaved projection work: exactly ONE half-kc or v-pair
            # unit per tile (~1.7us PE + ~1us ACT copy) so no tile gets a
            # lump; deadlines: kc_n before tile 4n, v-pair i before
            # attn(2i) at loop 2i+4
            KP = {2: (1, 0), 3: (1, 1), 6: (2, 0), 7: (2, 1), 10: (3, 0), 11: (3, 1)}
            VP = {1: 0, 4: 1, 5: 2, 8: 3, 9: 4, 12: 5, 13: 6, 14: 7}
            if t in KP:
                k_proj_mh(*KP[t])
            if t in VP:
                v_proj_pair(VP[t], on_dve=False)

        while pending:
            emit_attn(*pending.pop(0))

        for dc in range(4):
            if dc % 2 == 0:
                nc.vector.tensor_copy(attnT[:, dc, :], at[dc][:, :QI])
            else:
                nc.scalar.copy(attnT[:, dc, :], at[dc][:, :QI])

        # ---------------- output projection ----------------
        for mh in range(2):
            ot = ps.tile([NC_, 2, 512], fp32, tag="sc")
            for j in range(2):
                m = 2 * mh + j
                if with_bias:
                    bias_mm(ot[:, j, :], "bo", None)
                for c in range(CH):
                    nc.tensor.matmul(
                        ot[:, j, :],
                        lhsT=attnT[:, c, m * 128 : (m + 1) * 128],
                        rhs=wsb["wot"][:, c, :],
                        start=(c == 0 and not with_bias),
                        stop=(c == CH - 1),
                    )
            if mh == 0:
                nc.scalar.copy(outsb[:, 0:2, :], ot)
            else:
                nc.vector.tensor_copy(outsb[:, 2:4, :], ot)
            nc.sync.dma_start(
                out=out_d.rearrange("(m p) o -> p m o", p=NC_)[:, 2 * mh : 2 * mh + 2, :],
                in_=outsb[:, 2 * mh : 2 * mh + 2, :],
            )

    nc.compile()
    return nc


_CACHE = {}


def kernel(Q, K, V, w_q, b_q, w_k, b_k, w_v, b_v, w_o, b_o, _trace=False):
    import ml_dtypes
    from concourse import bass_utils

    bf = ml_dtypes.bfloat16
    Q = np.asarray(Q, np.float32)
    K = np.asarray(K, np.float32)
    V = np.asarray(V, np.float32)
    with_bias = any(
        np.any(np.asarray(b) != 0) for b in (b_q, b_k, b_v, b_o)
    )

    if ("nc", with_bias) not in _CACHE:
        _CACHE[("nc", with_bias)] = _build(with_bias)
    nc = _CACHE[("nc", with_bias)]

    wmaps = {
        "wqt": _chunk(np.asarray(w_q, np.float32).T, bf),
        "wkt": _chunk(np.asarray(w_k, np.float32).T, bf),
        "wvt": _chunk(np.asarray(w_v, np.float32).T, bf),
        "wot": _chunk(np.asarray(w_o, np.float32).T, bf),
        "ident": np.eye(NC_, dtype=bf),
    }
    if with_bias:
        for n, b in (("bq", b_q), ("bk", b_k), ("bv", b_v), ("bo", b_o)):
            wmaps[n] = np.ascontiguousarray(
                np.asarray(b, np.float32).reshape(1, D)
            ).astype(bf)

    in_maps = []
    for c in range(NCORES):
        b = c // CPB
        s0 = (c % CPB) * QI
        kt = _chunk(K[b].T, bf)                   # [128, 4c, 2048]
        vt = _chunk(V[b].T, bf)
        in_maps.append(
            dict(
                wmaps,
                qt=_chunk(Q[b, s0 : s0 + QI, :].T, bf),
                # [128, c, kc*512] -> [128, kc, c, 512]
                kt=np.ascontiguousarray(
                    kt.reshape(NC_, CH, NKC, 512).transpose(0, 2, 1, 3)
                ),
                # [128, c, t*128] -> [128, t, c, 128]
                vt=np.ascontiguousarray(
                    vt.reshape(NC_, CH, NKJ, KJT).transpose(0, 2, 1, 3)
                ),
            )
        )

    res = bass_utils.run_bass_kernel_spmd(
        nc, in_maps, core_ids=list(range(NCORES)), trace=_trace
    )

    out = np.empty((B, S, D), np.float32)
    for c in range(NCORES):
        b = c // CPB
        s0 = (c % CPB) * QI
        out[b, s0 : s0 + QI, :] = res.results[c]["out"]
    if _trace:
        kernel._last_results = res
    return out


# revision 33
# speedup vs baseline: 1.1573x; 1.0057x over previous
"""Multi-head attention (softmax over the HEADS axis) on 8 trn2 NeuronCores.

Reference math (B=2, S=2048, D=512, H=8, Dk=64):
    q = split_heads(Q @ w_q.T + b_q)          # [B,H,S,Dk]
    scores = q @ k.T / sqrt(Dk)               # [B,H,Sq,Sk]
    probs = softmax(scores, axis=1)           # softmax over H (source quirk!)
    attn = probs @ v                          # [B,H,Sq,Dk]
    out = concat_heads(attn) @ w_o.T + b_o    # [B,S,D]

Softmax over H is local to each (b, sq, sk) position: sharding over
(batch x query rows) needs no cross-core communication.  Core c handles
batch c//4, query rows (c%4)*512 .. +512, with all 8 heads resident.

Schedule: DMA is issued in dependency-priority order (wq,q / wk,k-chunks /
wv,v-chunks / wo) so the Q projection starts ~3us in and the first score
tile ~7us in.  K projection is emitted kc-major; kc1..3 and the V
projection pairs are interleaved into the attention loop so PE never has
a >1us bubble (HAM clock-gate stays at 8/8).  Score matmul pairs and attn
matmul pairs are emitted adjacently: they row/col-tile into disjoint PE
quadrant groups and overlap ~2x.

Steady state is elementwise-bound.  Measured: DVE tensor_tensor bf16
runs in 2x_1P mode (2 elem/cycle) ONLY when GpSimd is idle - any GpSimd
op degrades concurrent DVE ops 2-4x via the shared SBUF port.  So the
whole softmax chain lives on DVE (~5.3us/tile, all bf16 2x ops:
head-sum tree FD2048/FD1024, fp32 level-3 add, fast reciprocal, one
FD4096 broadcast normalization mul) and GpSimd is NOT used at all.
ACT does 4 exps [128,2,512] from PSUM (~4.0us/tile) plus all psum->sbuf
projection drains.  PSUM: 4 banks of score tiles (double-buffered,
shared with V/Q/K/O-proj pairs) + 4 banks of attn accumulators.
"""

import numpy as np

B, S, D, H, DK = 2, 2048, 512, 8, 64
NCORES = 8
CPB = NCORES // B          # cores per batch
QI = S // CPB              # query rows per core (512)
KJT = 128                  # kj tile (partition dim of scores)
NKJ = S // KJT             # 16 kj tiles
NC_, CH = 128, 4           # partitions, din chunks
NKC = 4                    # kj column chunks for K proj (512 each)
SCALE = 1.0 / np.sqrt(DK)  # folded into exp activation
LAG = 3                    # attn matmuls run LAG tiles behind softmax


def _chunk(x, dt):
    """[512, F] -> [128, 4, F] with row = chunk*128 + p."""
    f = x.shape[1]
    return np.ascontiguousarray(
        np.ascontiguousarray(x).reshape(CH, NC_, f).transpose(1, 0, 2)
    ).astype(dt)


def _build(with_bias):
    from contextlib import ExitStack

    import concourse.bass as bass
    import concourse.mybir as mybir
    import concourse.tile as tile
    from concourse import bacc
    from concourse.dve_ops import (
        RECIP_APPROX_FAST_CONSTS as _RC,
        RECIPROCAL_APPROX_FAST as _RF,
    )

    fp32 = mybir.dt.float32
    bf16 = mybir.dt.bfloat16
    EXP = mybir.ActivationFunctionType.Exp

    nc = bacc.Bacc(
        "TRN2",
        target_bir_lowering=False,
        debug=False,
        enable_asserts=False,
        num_devices=NCORES,
    )

    def din(name, shape):
        return nc.dram_tensor(name, shape, bf16, kind="ExternalInput").ap()

    qt_d = din("qt", [NC_, CH, QI])
    kt_d = din("kt", [NC_, NKC, CH, 512])    # kc-major so kc0 lands first
    vt_d = din("vt", [NC_, NKJ, CH, KJT])    # tile-major so early tiles land first
    id_d = din("ident", [NC_, NC_])          # identity for PE head-sum matmuls
    w_d = {n: din(n, [NC_, CH, D]) for n in ("wqt", "wkt", "wvt", "wot")}
    if with_bias:
        b_d = {n: din(n, [1, D]) for n in ("bq", "bk", "bv", "bo")}
    # p-major layout: out[p, m, :] is 4KB-contiguous per partition, so the
    # final DMA uses large descriptors (host transposes back, which is cheap)
    out_d = nc.dram_tensor("out", [NC_, CH, D], fp32, kind="ExternalOutput").ap()

    with tile.TileContext(nc) as tc, ExitStack() as ctx:
        wpool = ctx.enter_context(tc.tile_pool(name="wts", bufs=1))
        raw = ctx.enter_context(tc.tile_pool(name="raw", bufs=1))
        acts = ctx.enter_context(tc.tile_pool(name="acts", bufs=1))
        sm = ctx.enter_context(tc.tile_pool(name="sm", bufs=3))
        pp = ctx.enter_context(tc.tile_pool(name="pp", bufs=5))
        ps = ctx.enter_context(tc.tile_pool(name="ps", bufs=2, space="PSUM"))
        psa = ctx.enter_context(tc.tile_pool(name="psa", bufs=4, space="PSUM"))

        # ---------------- DMA in priority order ----------------
        wsb = {}
        for n in ("wqt", "wkt", "wvt", "wot"):
            wsb[n] = wpool.tile([NC_, CH, D], bf16, tag=n, name=n)
        qraw = raw.tile([NC_, CH, QI], bf16, tag="qraw")
        kraw = raw.tile([NC_, NKC, CH, 512], bf16, tag="kraw")
        vraw = raw.tile([NC_, NKJ, CH, KJT], bf16, tag="vraw")

        ident = acts.tile([NC_, NC_], bf16, tag="ident")
        nc.sync.dma_start(out=ident, in_=id_d)
        nc.sync.dma_start(out=wsb["wqt"], in_=w_d["wqt"])
        nc.sync.dma_start(out=qraw, in_=qt_d)
        nc.sync.dma_start(out=wsb["wkt"], in_=w_d["wkt"])
        nc.sync.dma_start(out=kraw[:, 0], in_=kt_d[:, 0])
        nc.sync.dma_start(out=kraw[:, 1], in_=kt_d[:, 1])
        nc.sync.dma_start(out=wsb["wvt"], in_=w_d["wvt"])
        nc.sync.dma_start(out=vraw[:, 0:8], in_=vt_d[:, 0:8])
        nc.sync.dma_start(out=kraw[:, 2], in_=kt_d[:, 2])
        nc.sync.dma_start(out=vraw[:, 8:16], in_=vt_d[:, 8:16])
        nc.sync.dma_start(out=kraw[:, 3], in_=kt_d[:, 3])
        nc.sync.dma_start(out=wsb["wot"], in_=w_d["wot"])

        if with_bias:
            ones = acts.tile([1, D], bf16, tag="ones")
            nc.vector.memset(ones, 1.0)
            brow = {}
            for n in ("bq", "bk", "bv", "bo"):
                brow[n] = acts.tile([1, D], bf16, tag=n, name=n)
                nc.sync.dma_start(out=brow[n], in_=b_d[n])

        def bias_mm(pt_ap, bname, col_slice):
            """rank-1 bias init: psum = bias-row (x) ones-row (or flipped)."""
            if col_slice is not None:  # bias along partitions
                lhsT = brow[bname][:, col_slice]
                rhs = ones[:, : pt_ap.shape[-1]]
            else:  # bias along free dim
                lhsT = ones[:, :128]
                rhs = brow[bname]
            nc.tensor.matmul(pt_ap, lhsT=lhsT, rhs=rhs, start=True, stop=False)

        qTs = acts.tile([NC_, CH, QI], bf16, tag="qTs")
        kTs = acts.tile([NC_, CH, S], bf16, tag="kTs")
        vs = acts.tile([NC_, NKJ, D], bf16, tag="vs")
        attnT = acts.tile([NC_, CH, QI], bf16, tag="attnT")
        outsb = acts.tile([NC_, CH, D], fp32, tag="outsb")

        at = [psa.tile([NC_, 512], fp32, tag="attn", name=f"at{i}") for i in range(4)]

        # PE warm-up: ~48 tiny matmuls as soon as `ident` lands, while the
        # big input DMAs stream.  Sustained PE activity releases the HAM
        # clock gate (4/8 -> 8/8) so the projections run at 2.4 GHz instead
        # of 1.2 GHz.  The throwaway psum tile has no readers and rotates
        # back into the scores pool, whose users all write with start=True.
        warm = ps.tile([NC_, 2, 512], fp32, tag="sc")
        for i in range(48):
            nc.tensor.matmul(
                warm[:, 0, 0:128], lhsT=ident, rhs=ident,
                start=True, stop=True, skip_group_check=True,
            )

        # ---------------- Q projection ----------------
        for mh in range(2):  # two m per psum tile
            pt = ps.tile([NC_, 2, 512], fp32, tag="sc")
            for j in range(2):
                m = 2 * mh + j
                if with_bias:
                    bias_mm(pt[:, j, :QI], "bq", slice(m * 128, (m + 1) * 128))
                for c in range(CH):
                    nc.tensor.matmul(
                        pt[:, j, :QI],
                        lhsT=wsb["wqt"][:, c, m * 128 : (m + 1) * 128],
                        rhs=qraw[:, c, :],
                        start=(c == 0 and not with_bias),
                        stop=(c == CH - 1),
                    )
            nc.scalar.copy(qTs[:, 2 * mh : 2 * mh + 2, :], pt)

        def k_proj_mh(kc, mh):
            pt = ps.tile([NC_, 2, 512], fp32, tag="sc")
            for j in range(2):
                m = 2 * mh + j
                if with_bias:
                    bias_mm(pt[:, j, :], "bk", slice(m * 128, (m + 1) * 128))
                for c in range(CH):
                    nc.tensor.matmul(
                        pt[:, j, :],
                        lhsT=wsb["wkt"][:, c, m * 128 : (m + 1) * 128],
                        rhs=kraw[:, kc, c, :],
                        start=(c == 0 and not with_bias),
                        stop=(c == CH - 1),
                    )
            nc.scalar.copy(
                kTs[:, 2 * mh : 2 * mh + 2, kc * 512 : (kc + 1) * 512], pt
            )

        k_proj_mh(0, 0)
        k_proj_mh(0, 1)

        def v_proj_pair(i, on_dve):
            """project v tiles 2i, 2i+1 into vs (one psum tile)."""
            pt = ps.tile([NC_, 2, 512], fp32, tag="sc")
            for half in range(2):
                t = 2 * i + half
                if with_bias:
                    bias_mm(pt[:, half, :], "bv", None)
                for c in range(CH):
                    nc.tensor.matmul(
                        pt[:, half, :],
                        lhsT=vraw[:, t, c, :],
                        rhs=wsb["wvt"][:, c, :],
                        start=(c == 0 and not with_bias),
                        stop=(c == CH - 1),
                    )
            if on_dve:
                nc.vector.tensor_copy(vs[:, 2 * i : 2 * i + 2, :], pt[:, :, :])
            else:
                nc.scalar.copy(vs[:, 2 * i : 2 * i + 2, :], pt[:, :, :])

        # attn psum: tile dc holds heads 2dc (p 0..63), 2dc+1 (p 64..127)
        def emit_attn(td, prs):
            for h in range(H):
                po = (h % 2) * 64
                nc.tensor.matmul(
                    at[h // 2][po : po + 64, :QI],
                    lhsT=vs[:, td, h * 64 : (h + 1) * 64],
                    rhs=prs[:, h, :],
                    start=(td == 0),
                    stop=(td == NKJ - 1),
                )

        # ---------------- attention loop ----------------
        pending = []

        def softmax_tail(tp, exp_p):
            """Head-sum tree + recip + normalize, all on DVE: bf16
            tensor_tensor hits 2x_1P mode (GpSimd stays idle to preserve
            it - shared SBUF port).  Level 1 is split so its first half
            only needs exps m0/m1 and runs in the shadow of exps m2/m3."""
            a1 = sm.tile([NC_, 4, QI], bf16, tag="a1")
            nc.vector.tensor_add(a1[:, 0:2, :], exp_p[:, 0:2, :], exp_p[:, 2:4, :])
            nc.vector.tensor_add(a1[:, 2:4, :], exp_p[:, 4:6, :], exp_p[:, 6:8, :])
            a2 = sm.tile([NC_, 2, QI], bf16, tag="a2")
            nc.vector.tensor_add(a2, a1[:, 0:2, :], a1[:, 2:4, :])
            ssum = sm.tile([NC_, QI], fp32, tag="ssum")
            nc.vector.tensor_add(ssum, a2[:, 0, :], a2[:, 1, :])
            r = sm.tile([NC_, QI], bf16, tag="r")
            nc.vector._custom_dve(
                _RF, out=r, in0=ssum, s0=_RC["s0"], s1=_RC["s1"], imm2=_RC["imm2"]
            )
            pr = pp.tile([NC_, H, QI], bf16, tag="probs")
            if tp == NKJ - 1:
                # last tile: split so attn(15) heads 0-3 start one mul earlier
                rb4 = r.unsqueeze(1).broadcast_to([NC_, 4, QI])
                nc.vector.tensor_mul(pr[:, 0:4, :], exp_p[:, 0:4, :], rb4)
                nc.vector.tensor_mul(pr[:, 4:8, :], exp_p[:, 4:8, :], rb4)
            else:
                rb = r.unsqueeze(1).broadcast_to([NC_, H, QI])
                nc.vector.tensor_mul(pr, exp_p, rb)
            pending.append((tp, pr))

        for t in range(NKJ):
            exp_t = sm.tile([NC_, H, QI], bf16, tag="exp")
            for m in range(4):
                spt = ps.tile([NC_, 2, 512], fp32, tag="sc")
                for j in range(2):
                    po = j * 64
                    nc.tensor.matmul(
                        spt[:, j, :QI],
                        lhsT=kTs[po : po + 64, m, t * 128 : (t + 1) * 128],
                        rhs=qTs[po : po + 64, m, :],
                        start=True,
                        stop=True,
                    )
                nc.scalar.activation(
                    exp_t[:, 2 * m : 2 * m + 2, :], spt, EXP, scale=SCALE
                )

            # pop BEFORE this tile's append: attn(td) emits at iter td+3,
            # strictly after the v-pair writing vs[td] (VP deadlines below)
            if len(pending) >= LAG:
                emit_attn(*pending.pop(0))
            softmax_tail(t, exp_t)
            # interleaved projection work: exactly ONE half-kc or v-pair
            # unit per tile (~1.7us PE + ~1us ACT copy) so no tile gets a
            # lump; deadlines: kc_n before tile 4n, v-pair i before
            # attn(2i) at loop 2i+4
            KP = {2: (1, 0), 3: (1, 1), 6: (2, 0), 7: (2, 1), 10: (3, 0), 11: (3, 1)}
            VP = {1: 0, 4: 1, 5: 2, 8: 3, 9: 4, 12: 5, 13: 6, 14: 7}
            if t in KP:
                k_proj_mh(*KP[t])
            if t in VP:
                v_proj_pair(VP[t], on_dve=False)

        # drain attn 13/14 whose probs are ready, then keep PE's HAM clock
        # warm with throwaway matmuls while DVE finishes tile 15's softmax
        # chain, so attn(15) + the output projection run at 2.4 GHz
        while len(pending) > 1:
            emit_attn(*pending.pop(0))
        warm2 = ps.tile([NC_, 2, 512], fp32, tag="sc")
        for i in range(40):
            nc.tensor.matmul(
                warm2[:, 0, 0:128], lhsT=ident, rhs=ident,
                start=True, stop=True, skip_group_check=True,
            )
        emit_attn(*pending.pop(0))

        for dc in range(4):
            if dc % 2 == 0:
                nc.vector.tensor_copy(attnT[:, dc, :], at[dc][:, :QI])
            else:
                nc.scalar.copy(attnT[:, dc, :], at[dc][:, :QI])

        # ---------------- output projection ----------------
        for mh in range(2):
            ot = ps.tile([NC_, 2, 512], fp32, tag="sc")
            for j in range(2):
                m = 2 * mh + j
                if with_bias:
                    bias_mm(ot[:, j, :], "bo", None)
                for c in range(CH):
                    nc.tensor.matmul(
                        ot[:, j, :],
                        lhsT=attnT[:, c, m * 128 : (m + 1) * 128],
                        rhs=wsb["wot"][:, c, :],
                        start=(c == 0 and not with_bias),
                        stop=(c == CH - 1),
                    )
            for j in range(2):
                m = 2 * mh + j
                if m % 2 == 0:
                    nc.scalar.copy(outsb[:, m, :], ot[:, j, :])
                else:
                    nc.vector.tensor_copy(outsb[:, m, :], ot[:, j, :])
                nc.sync.dma_start(out=out_d[:, m, :], in_=outsb[:, m, :])

    nc.compile()
    return nc


_CACHE = {}


def kernel(Q, K, V, w_q, b_q, w_k, b_k, w_v, b_v, w_o, b_o, _trace=False):
    import ml_dtypes
    from concourse import bass_utils

    bf = ml_dtypes.bfloat16
    Q = np.asarray(Q, np.float32)
    K = np.asarray(K, np.float32)
    V = np.asarray(V, np.float32)
    with_bias = any(
        np.any(np.asarray(b) != 0) for b in (b_q, b_k, b_v, b_o)
    )

    if ("nc", with_bias) not in _CACHE:
        _CACHE[("nc", with_bias)] = _build(with_bias)
    nc = _CACHE[("nc", with_bias)]

    wmaps = {
        "wqt": _chunk(np.asarray(w_q, np.float32).T, bf),
        "wkt": _chunk(np.asarray(w_k, np.float32).T, bf),
        "wvt": _chunk(np.asarray(w_v, np.float32).T, bf),
        "wot": _chunk(np.asarray(w_o, np.float32).T, bf),
        "ident": np.eye(NC_, dtype=bf),
    }
    if with_bias:
        for n, b in (("bq", b_q), ("bk", b_k), ("bv", b_v), ("bo", b_o)):
            wmaps[n] = np.ascontiguousarray(
                np.asarray(b, np.float32).reshape(1, D)
            ).astype(bf)

    in_maps = []
    for c in range(NCORES):
        b = c // CPB
        s0 = (c % CPB) * QI
        kt = _chunk(K[b].T, bf)                   # [128, 4c, 2048]
        vt = _chunk(V[b].T, bf)
        in_maps.append(
            dict(
                wmaps,
                qt=_chunk(Q[b, s0 : s0 + QI, :].T, bf),
                # [128, c, kc*512] -> [128, kc, c, 512]
                kt=np.ascontiguousarray(
                    kt.reshape(NC_, CH, NKC, 512).transpose(0, 2, 1, 3)
                ),
                # [128, c, t*128] -> [128, t, c, 128]
                vt=np.ascontiguousarray(
                    vt.reshape(NC_, CH, NKJ, KJT).transpose(0, 2, 1, 3)
                ),
            )
        )

    res = bass_utils.run_bass_kernel_spmd(
        nc, in_maps, core_ids=list(range(NCORES)), trace=_trace
    )

    out = np.empty((B, S, D), np.float32)
    for c in range(NCORES):
        b = c // CPB
        s0 = (c % CPB) * QI
        # device layout is [p, m, o]; row s0 + m*128 + p holds out[p, m, :]
        out[b, s0 : s0 + QI, :] = (
            np.asarray(res.results[c]["out"]).transpose(1, 0, 2).reshape(QI, D)
        )
    if _trace:
        kernel._last_results = res
    return out


# revision 35
# speedup vs baseline: 1.1654x; 1.0071x over previous
"""Multi-head attention (softmax over the HEADS axis) on 8 trn2 NeuronCores.

Reference math (B=2, S=2048, D=512, H=8, Dk=64):
    q = split_heads(Q @ w_q.T + b_q)          # [B,H,S,Dk]
    scores = q @ k.T / sqrt(Dk)               # [B,H,Sq,Sk]
    probs = softmax(scores, axis=1)           # softmax over H (source quirk!)
    attn = probs @ v                          # [B,H,Sq,Dk]
    out = concat_heads(attn) @ w_o.T + b_o    # [B,S,D]

Softmax over H is local to each (b, sq, sk) position: sharding over
(batch x query rows) needs no cross-core communication.  Core c handles
batch c//4, query rows (c%4)*512 .. +512, with all 8 heads resident.

Schedule: DMA is issued in dependency-priority order (wq,q / wk,k-chunks /
wv,v-chunks / wo) so the Q projection starts ~3us in and the first score
tile ~7us in.  K projection is emitted kc-major; kc1..3 and the V
projection pairs are interleaved into the attention loop so PE never has
a >1us bubble (HAM clock-gate stays at 8/8).  Score matmul pairs and attn
matmul pairs are emitted adjacently: they row/col-tile into disjoint PE
quadrant groups and overlap ~2x.

Steady state is elementwise-bound.  Measured: DVE tensor_tensor bf16
runs in 2x_1P mode (2 elem/cycle) ONLY when GpSimd is idle - any GpSimd
op degrades concurrent DVE ops 2-4x via the shared SBUF port.  So the
whole softmax chain lives on DVE (~5.3us/tile, all bf16 2x ops:
head-sum tree FD2048/FD1024, fp32 level-3 add, fast reciprocal, one
FD4096 broadcast normalization mul) and GpSimd is NOT used at all.
ACT does 4 exps [128,2,512] from PSUM (~4.0us/tile) plus all psum->sbuf
projection drains.  PSUM: 4 banks of score tiles (double-buffered,
shared with V/Q/K/O-proj pairs) + 4 banks of attn accumulators.
"""

import numpy as np

B, S, D, H, DK = 2, 2048, 512, 8, 64
NCORES = 8
CPB = NCORES // B          # cores per batch
QI = S // CPB              # query rows per core (512)
KJT = 128                  # kj tile (partition dim of scores)
NKJ = S // KJT             # 16 kj tiles
NC_, CH = 128, 4           # partitions, din chunks
NKC = 4                    # kj column chunks for K proj (512 each)
SCALE = 1.0 / np.sqrt(DK)  # folded into exp activation
LAG = 3                    # attn matmuls run LAG tiles behind softmax


def _chunk(x, dt):
    """[512, F] -> [128, 4, F] with row = chunk*128 + p."""
    f = x.shape[1]
    return np.ascontiguousarray(
        np.ascontiguousarray(x).reshape(CH, NC_, f).transpose(1, 0, 2)
    ).astype(dt)


def _build(with_bias):
    from contextlib import ExitStack

    import concourse.bass as bass
    import concourse.mybir as mybir
    import concourse.tile as tile
    from concourse import bacc
    from concourse.dve_ops import (
        RECIP_APPROX_FAST_CONSTS as _RC,
        RECIPROCAL_APPROX_FAST as _RF,
    )

    fp32 = mybir.dt.float32
    bf16 = mybir.dt.bfloat16
    EXP = mybir.ActivationFunctionType.Exp

    nc = bacc.Bacc(
        "TRN2",
        target_bir_lowering=False,
        debug=False,
        enable_asserts=False,
        num_devices=NCORES,
    )

    def din(name, shape):
        return nc.dram_tensor(name, shape, bf16, kind="ExternalInput").ap()

    qt_d = din("qt", [NC_, CH, QI])
    kt_d = din("kt", [NC_, NKC, CH, 512])    # kc-major so kc0 lands first
    vt_d = din("vt", [NC_, NKJ, CH, KJT])    # tile-major so early tiles land first
    id_d = din("ident", [NC_, NC_])          # identity for PE head-sum matmuls
    w_d = {n: din(n, [NC_, CH, D]) for n in ("wqt", "wkt", "wvt", "wot")}
    if with_bias:
        b_d = {n: din(n, [1, D]) for n in ("bq", "bk", "bv", "bo")}
    # p-major layout: out[p, m, :] is 4KB-contiguous per partition, so the
    # final DMA uses large descriptors (host transposes back, which is cheap)
    out_d = nc.dram_tensor("out", [NC_, CH, D], fp32, kind="ExternalOutput").ap()

    with tile.TileContext(nc) as tc, ExitStack() as ctx:
        wpool = ctx.enter_context(tc.tile_pool(name="wts", bufs=1))
        raw = ctx.enter_context(tc.tile_pool(name="raw", bufs=1))
        acts = ctx.enter_context(tc.tile_pool(name="acts", bufs=1))
        sm = ctx.enter_context(tc.tile_pool(name="sm", bufs=3))
        pp = ctx.enter_context(tc.tile_pool(name="pp", bufs=5))
        ps = ctx.enter_context(tc.tile_pool(name="ps", bufs=2, space="PSUM"))
        psa = ctx.enter_context(tc.tile_pool(name="psa", bufs=4, space="PSUM"))

        # ---------------- DMA in priority order ----------------
        wsb = {}
        for n in ("wqt", "wkt", "wvt", "wot"):
            wsb[n] = wpool.tile([NC_, CH, D], bf16, tag=n, name=n)
        qraw = raw.tile([NC_, CH, QI], bf16, tag="qraw")
        kraw = raw.tile([NC_, NKC, CH, 512], bf16, tag="kraw")
        vraw = raw.tile([NC_, NKJ, CH, KJT], bf16, tag="vraw")

        ident = acts.tile([NC_, NC_], bf16, tag="ident")
        nc.sync.dma_start(out=ident, in_=id_d)
        nc.sync.dma_start(out=wsb["wqt"], in_=w_d["wqt"])
        nc.sync.dma_start(out=qraw, in_=qt_d)
        nc.sync.dma_start(out=wsb["wkt"], in_=w_d["wkt"])
        nc.sync.dma_start(out=kraw[:, 0], in_=kt_d[:, 0])
        nc.sync.dma_start(out=kraw[:, 1], in_=kt_d[:, 1])
        nc.sync.dma_start(out=wsb["wvt"], in_=w_d["wvt"])
        nc.sync.dma_start(out=vraw[:, 0:8], in_=vt_d[:, 0:8])
        nc.sync.dma_start(out=kraw[:, 2], in_=kt_d[:, 2])
        nc.sync.dma_start(out=vraw[:, 8:16], in_=vt_d[:, 8:16])
        nc.sync.dma_start(out=kraw[:, 3], in_=kt_d[:, 3])
        nc.sync.dma_start(out=wsb["wot"], in_=w_d["wot"])

        if with_bias:
            ones = acts.tile([1, D], bf16, tag="ones")
            nc.vector.memset(ones, 1.0)
            brow = {}
            for n in ("bq", "bk", "bv", "bo"):
                brow[n] = acts.tile([1, D], bf16, tag=n, name=n)
                nc.sync.dma_start(out=brow[n], in_=b_d[n])

        def bias_mm(pt_ap, bname, col_slice):
            """rank-1 bias init: psum = bias-row (x) ones-row (or flipped)."""
            if col_slice is not None:  # bias along partitions
                lhsT = brow[bname][:, col_slice]
                rhs = ones[:, : pt_ap.shape[-1]]
            else:  # bias along free dim
                lhsT = ones[:, :128]
                rhs = brow[bname]
            nc.tensor.matmul(pt_ap, lhsT=lhsT, rhs=rhs, start=True, stop=False)

        qTs = acts.tile([NC_, CH, QI], bf16, tag="qTs")
        kTs = acts.tile([NC_, CH, S], bf16, tag="kTs")
        vs = acts.tile([NC_, NKJ, D], bf16, tag="vs")
        attnT = acts.tile([NC_, CH, QI], bf16, tag="attnT")
        outsb = acts.tile([NC_, CH, D], fp32, tag="outsb")

        at = [psa.tile([NC_, 512], fp32, tag="attn", name=f"at{i}") for i in range(4)]

        # PE warm-up: ~48 tiny matmuls as soon as `ident` lands, while the
        # big input DMAs stream.  Sustained PE activity releases the HAM
        # clock gate (4/8 -> 8/8) so the projections run at 2.4 GHz instead
        # of 1.2 GHz.  The throwaway psum tile has no readers and rotates
        # back into the scores pool, whose users all write with start=True.
        warm = ps.tile([NC_, 2, 512], fp32, tag="sc")
        for i in range(48):
            nc.tensor.matmul(
                warm[:, 0, 0:128], lhsT=ident, rhs=ident,
                start=True, stop=True, skip_group_check=True,
            )

        # ---------------- Q projection ----------------
        for mh in range(2):  # two m per psum tile
            pt = ps.tile([NC_, 2, 512], fp32, tag="sc")
            for j in range(2):
                m = 2 * mh + j
                if with_bias:
                    bias_mm(pt[:, j, :QI], "bq", slice(m * 128, (m + 1) * 128))
                for c in range(CH):
                    nc.tensor.matmul(
                        pt[:, j, :QI],
                        lhsT=wsb["wqt"][:, c, m * 128 : (m + 1) * 128],
                        rhs=qraw[:, c, :],
                        start=(c == 0 and not with_bias),
                        stop=(c == CH - 1),
                    )
            # first drain on DVE (idle at startup) so ACT reaches the first
            # exp sooner; ACT is the steady-state exp engine
            if mh == 0:
                nc.vector.tensor_copy(qTs[:, 0:2, :], pt)
            else:
                nc.scalar.copy(qTs[:, 2:4, :], pt)

        def k_proj_mh(kc, mh):
            pt = ps.tile([NC_, 2, 512], fp32, tag="sc")
            for j in range(2):
                m = 2 * mh + j
                if with_bias:
                    bias_mm(pt[:, j, :], "bk", slice(m * 128, (m + 1) * 128))
                for c in range(CH):
                    nc.tensor.matmul(
                        pt[:, j, :],
                        lhsT=wsb["wkt"][:, c, m * 128 : (m + 1) * 128],
                        rhs=kraw[:, kc, c, :],
                        start=(c == 0 and not with_bias),
                        stop=(c == CH - 1),
                    )
            dst = kTs[:, 2 * mh : 2 * mh + 2, kc * 512 : (kc + 1) * 512]
            if kc == 0 and mh == 0:
                nc.vector.tensor_copy(dst, pt)
            else:
                nc.scalar.copy(dst, pt)

        k_proj_mh(0, 0)
        k_proj_mh(0, 1)

        def v_proj_pair(i, on_dve):
            """project v tiles 2i, 2i+1 into vs (one psum tile)."""
            pt = ps.tile([NC_, 2, 512], fp32, tag="sc")
            for half in range(2):
                t = 2 * i + half
                if with_bias:
                    bias_mm(pt[:, half, :], "bv", None)
                for c in range(CH):
                    nc.tensor.matmul(
                        pt[:, half, :],
                        lhsT=vraw[:, t, c, :],
                        rhs=wsb["wvt"][:, c, :],
                        start=(c == 0 and not with_bias),
                        stop=(c == CH - 1),
                    )
            if on_dve:
                nc.vector.tensor_copy(vs[:, 2 * i : 2 * i + 2, :], pt[:, :, :])
            else:
                nc.scalar.copy(vs[:, 2 * i : 2 * i + 2, :], pt[:, :, :])

        # attn psum: tile dc holds heads 2dc (p 0..63), 2dc+1 (p 64..127)
        def emit_attn(td, prs):
            for h in range(H):
                po = (h % 2) * 64
                nc.tensor.matmul(
                    at[h // 2][po : po + 64, :QI],
                    lhsT=vs[:, td, h * 64 : (h + 1) * 64],
                    rhs=prs[:, h, :],
                    start=(td == 0),
                    stop=(td == NKJ - 1),
                )

        # ---------------- attention loop ----------------
        pending = []

        def softmax_tail(tp, exp_p):
            """Head-sum tree + recip + normalize, all on DVE: bf16
            tensor_tensor hits 2x_1P mode (GpSimd stays idle to preserve
            it - shared SBUF port).  Level 1 is split so its first half
            only needs exps m0/m1 and runs in the shadow of exps m2/m3."""
            a1 = sm.tile([NC_, 4, QI], bf16, tag="a1")
            nc.vector.tensor_add(a1[:, 0:2, :], exp_p[:, 0:2, :], exp_p[:, 2:4, :])
            nc.vector.tensor_add(a1[:, 2:4, :], exp_p[:, 4:6, :], exp_p[:, 6:8, :])
            a2 = sm.tile([NC_, 2, QI], bf16, tag="a2")
            nc.vector.tensor_add(a2, a1[:, 0:2, :], a1[:, 2:4, :])
            ssum = sm.tile([NC_, QI], fp32, tag="ssum")
            nc.vector.tensor_add(ssum, a2[:, 0, :], a2[:, 1, :])
            r = sm.tile([NC_, QI], bf16, tag="r")
            nc.vector._custom_dve(
                _RF, out=r, in0=ssum, s0=_RC["s0"], s1=_RC["s1"], imm2=_RC["imm2"]
            )
            pr = pp.tile([NC_, H, QI], bf16, tag="probs")
            if tp == NKJ - 1:
                # last tile: split so attn(15) heads 0-3 start one mul earlier
                rb4 = r.unsqueeze(1).broadcast_to([NC_, 4, QI])
                nc.vector.tensor_mul(pr[:, 0:4, :], exp_p[:, 0:4, :], rb4)
                nc.vector.tensor_mul(pr[:, 4:8, :], exp_p[:, 4:8, :], rb4)
            else:
                rb = r.unsqueeze(1).broadcast_to([NC_, H, QI])
                nc.vector.tensor_mul(pr, exp_p, rb)
            pending.append((tp, pr))

        for t in range(NKJ):
            exp_t = sm.tile([NC_, H, QI], bf16, tag="exp")
            for m in range(4):
                spt = ps.tile([NC_, 2, 512], fp32, tag="sc")
                for j in range(2):
                    po = j * 64
                    nc.tensor.matmul(
                        spt[:, j, :QI],
                        lhsT=kTs[po : po + 64, m, t * 128 : (t + 1) * 128],
                        rhs=qTs[po : po + 64, m, :],
                        start=True,
                        stop=True,
                    )
                nc.scalar.activation(
                    exp_t[:, 2 * m : 2 * m + 2, :], spt, EXP, scale=SCALE
                )

            # pop BEFORE this tile's append: attn(td) emits at iter td+3,
            # strictly after the v-pair writing vs[td] (VP deadlines below)
            if len(pending) >= LAG:
                emit_attn(*pending.pop(0))
            softmax_tail(t, exp_t)
            # interleaved projection work: exactly ONE half-kc or v-pair
            # unit per tile (~1.7us PE + ~1us ACT copy) so no tile gets a
            # lump; deadlines: kc_n before tile 4n, v-pair i before
            # attn(2i) at loop 2i+4
            KP = {2: (1, 0), 3: (1, 1), 6: (2, 0), 7: (2, 1), 10: (3, 0), 11: (3, 1)}
            VP = {1: 0, 4: 1, 5: 2, 8: 3, 9: 4, 12: 5, 13: 6, 14: 7}
            if t in KP:
                k_proj_mh(*KP[t])
            if t in VP:
                v_proj_pair(VP[t], on_dve=False)

        # drain attn 13/14 whose probs are ready, then keep PE's HAM clock
        # warm with throwaway matmuls while DVE finishes tile 15's softmax
        # chain, so attn(15) + the output projection run at 2.4 GHz
        while len(pending) > 1:
            emit_attn(*pending.pop(0))
        warm2 = ps.tile([NC_, 2, 512], fp32, tag="sc")
        for i in range(40):
            nc.tensor.matmul(
                warm2[:, 0, 0:128], lhsT=ident, rhs=ident,
                start=True, stop=True, skip_group_check=True,
            )
        emit_attn(*pending.pop(0))

        for dc in range(4):
            if dc % 2 == 0:
                nc.vector.tensor_copy(attnT[:, dc, :], at[dc][:, :QI])
            else:
                nc.scalar.copy(attnT[:, dc, :], at[dc][:, :QI])

        # ---------------- output projection ----------------
        for mh in range(2):
            ot = ps.tile([NC_, 2, 512], fp32, tag="sc")
            for j in range(2):
                m = 2 * mh + j
                if with_bias:
                    bias_mm(ot[:, j, :], "bo", None)
                for c in range(CH):
                    nc.tensor.matmul(
                        ot[:, j, :],
                        lhsT=attnT[:, c, m * 128 : (m + 1) * 128],
                        rhs=wsb["wot"][:, c, :],
                        start=(c == 0 and not with_bias),
                        stop=(c == CH - 1),
                    )
            for j in range(2):
                m = 2 * mh + j
                if m % 2 == 0:
                    nc.scalar.copy(outsb[:, m, :], ot[:, j, :])
                else:
                    nc.vector.tensor_copy(outsb[:, m, :], ot[:, j, :])
                nc.sync.dma_start(out=out_d[:, m, :], in_=outsb[:, m, :])

    nc.compile()
    return nc


_CACHE = {}


def kernel(Q, K, V, w_q, b_q, w_k, b_k, w_v, b_v, w_o, b_o, _trace=False):
    import ml_dtypes
    from concourse import bass_utils

    bf = ml_dtypes.bfloat16
    Q = np.asarray(Q, np.float32)
    K = np.asarray(K, np.float32)
    V = np.asarray(V, np.float32)
    with_bias = any(
        np.any(np.asarray(b) != 0) for b in (b_q, b_k, b_v, b_o)
    )

    if ("nc", with_bias) not in _CACHE:
        _CACHE[("nc", with_bias)] = _build(with_bias)
    nc = _CACHE[("nc", with_bias)]

    wmaps = {
        "wqt": _chunk(np.asarray(w_q, np.float32).T, bf),
        "wkt": _chunk(np.asarray(w_k, np.float32).T, bf),
        "wvt": _chunk(np.asarray(w_v, np.float32).T, bf),
        "wot": _chunk(np.asarray(w_o, np.float32).T, bf),
        "ident": np.eye(NC_, dtype=bf),
    }
    if with_bias:
        for n, b in (("bq", b_q), ("bk", b_k), ("bv", b_v), ("bo", b_o)):
            wmaps[n] = np.ascontiguousarray(
                np.asarray(b, np.float32).reshape(1, D)
            ).astype(bf)

    in_maps = []
    for c in range(NCORES):
        b = c // CPB
        s0 = (c % CPB) * QI
        kt = _chunk(K[b].T, bf)                   # [128, 4c, 2048]
        vt = _chunk(V[b].T, bf)
        in_maps.append(
            dict(
                wmaps,
                qt=_chunk(Q[b, s0 : s0 + QI, :].T, bf),
                # [128, c, kc*512] -> [128, kc, c, 512]
                kt=np.ascontiguousarray(
                    kt.reshape(NC_, CH, NKC, 512).transpose(0, 2, 1, 3)
                ),
                # [128, c, t*128] -> [128, t, c, 128]
                vt=np.ascontiguousarray(
                    vt.reshape(NC_, CH, NKJ, KJT).transpose(0, 2, 1, 3)
                ),
            )
        )

    res = bass_utils.run_bass_kernel_spmd(
        nc, in_maps, core_ids=list(range(NCORES)), trace=_trace
    )

    out = np.empty((B, S, D), np.float32)
    for c in range(NCORES):
        b = c // CPB
        s0 = (c % CPB) * QI
        # device layout is [p, m, o]; row s0 + m*128 + p holds out[p, m, :]
        out[b, s0 : s0 + QI, :] = (
            np.asarray(res.results[c]["out"]).transpose(1, 0, 2).reshape(QI, D)
        )
    if _trace:
        kernel._last_results = res
    return out
